# revision 30
# baseline (speedup 1.0000x reference)
"""MQA attention (LN + QKV proj + RoPE + causal attn + out-proj) on 8 trn2 cores.

Sharding: tensor-parallel over heads (2 heads/core, Wq cols + Wo rows), KV
replicated (single KV head), out-proj produces per-core partial sums that the
host reduces.

v4 per-core dataflow:
  Host ships BOTH x (natural, for LN stats) and xT (transposed, for matmul
  rhs) — no on-device activation transposes (DMA-xbar transposes serialize
  against all other DMA traffic, ~8.5us each; PE transposes burn the
  critical engine).
  LayerNorm is folded into the projections:
      proj(LN(x)) = (W^T x - csum * mu_t) * rstd_t
  with csum = W^T 1 precomputed on host, and mu/rstd per token applied at
  PSUM-evict time (rstd folded into the RoPE cos/sin for q,k).
  rstd comes from a Newton rsqrt on DVE (no ACT sqrt -> single exp table).
  Per-token stats cross from partition-layout to row-layout via a tiny
  DRAM bounce (gpsimd SWDGE), as does the softmax-sum reciprocal.
  Attention: S^T = k @ q^T causal-blocked -> exp on ScalarE -> AV + row
  sums via ones-matmul -> normalize -> out-proj partial -> host sum.
  Emission pipelines b0-attention with b1-projections and out-proj with
  attention to keep the PE dense (HAM stays warm).
"""

import sys

if "/opt/trn_rl_repo" not in sys.path:
    sys.path.insert(0, "/opt/trn_rl_repo")

import ml_dtypes
import numpy as np

import concourse.bass as bass
import concourse.tile as tile
from concourse import bacc, mybir
from concourse.masks import make_identity

F32 = mybir.dt.float32
DT = mybir.dt.bfloat16  # matmul operand storage dtype
DT_NP = ml_dtypes.bfloat16

B, N, DIM, DH, HEADS = 2, 2048, 2048, 128, 16
H_LOCAL = 2  # heads per core
N_CORES = 8
KT = DIM // 128  # k-tiles over the model dim
TT = N // 128  # token tiles per batch
CHUNK = 512  # token chunk for projection phase
NCH = N // CHUNK  # chunks per batch
QG = 512  # q-group width in attention
NQG = N // QG
SCALE = float(DH) ** -0.5
EPS = 1e-5
NEG = -1e30
WARM_MMS = 48


def build_nc(repeat=1):
    nc = bacc.Bacc(None, target_bir_lowering=False, debug=False)

    x_d = nc.dram_tensor("x_in", [B, N, DIM], DT, kind="ExternalInput")
    xt_d = nc.dram_tensor("xT", [128, B, NCH, KT, CHUNK], DT, kind="ExternalInput")
    wq_d = nc.dram_tensor("wq", [128, KT, H_LOCAL * DH], DT, kind="ExternalInput")
    wk_d = nc.dram_tensor("wk", [128, KT, DH], DT, kind="ExternalInput")
    wv_d = nc.dram_tensor("wv", [128, KT, DH], DT, kind="ExternalInput")
    wo_d = nc.dram_tensor("wo", [128, H_LOCAL, DIM], DT, kind="ExternalInput")
    csq_d = nc.dram_tensor("csq", [128, H_LOCAL], F32, kind="ExternalInput")
    csk_d = nc.dram_tensor("csk", [128, 1], F32, kind="ExternalInput")
    csv_d = nc.dram_tensor("csv", [128, 1], F32, kind="ExternalInput")
    cos_d = nc.dram_tensor("cosT", [DH, N], DT, kind="ExternalInput")
    sin_d = nc.dram_tensor("sinT", [DH, N], DT, kind="ExternalInput")
    msk_d = nc.dram_tensor("mask", [128, 128], DT, kind="ExternalInput")
    out_d = nc.dram_tensor("out_partial", [B, N, DIM], DT, kind="ExternalOutput")

    with tile.TileContext(nc) as tc:
        with (
            tc.tile_pool(name="const", bufs=1) as const,
            tc.tile_pool(name="xp", bufs=8) as xp,
            tc.tile_pool(name="xtp", bufs=2) as xtp,
            tc.tile_pool(name="store", bufs=1) as store,
            tc.tile_pool(name="small", bufs=6) as small,
            tc.tile_pool(name="rope", bufs=2) as ropep,
            tc.tile_pool(name="bc", bufs=2) as bcp,
            tc.tile_pool(name="ep", bufs=4) as ep,
            tc.tile_pool(name="sm", bufs=2) as sm,
            tc.tile_pool(name="vb", bufs=2) as vb,
            tc.tile_pool(name="op", bufs=3) as op,
            tc.tile_pool(name="dr", bufs=2, space="DRAM") as drp,
            tc.tile_pool(name="ps", bufs=1, space="PSUM") as ps,
        ):
            # --- first chunk's inputs before the weights so LN starts early ---
            pre_x = {}

            def load_x(b, cg):
                key = (b, cg)
                if key in pre_x:
                    return pre_x.pop(key)
                tiles = []
                c0 = cg * CHUNK
                for t in range(CHUNK // 128):
                    tok0 = c0 + t * 128
                    x_t = xp.tile([128, DIM], DT, tag="x", name=f"x_{b}_{cg}_{t}")
                    nc.gpsimd.dma_start(x_t[:], x_d[b, tok0 : tok0 + 128, :])
                    tiles.append(x_t)
                return tiles

            pre_xt = {}

            def load_xt(b, cg):
                key = (b, cg)
                if key in pre_xt:
                    return pre_xt.pop(key)
                xtc = xtp.tile([128, KT, CHUNK], DT, tag="xtc", name=f"xt_{b}_{cg}")
                nc.gpsimd.dma_start(xtc[:], xt_d[:, b, cg])
                return xtc

            pre_xt[(0, 0)] = load_xt(0, 0)
            pre_x[(0, 0)] = load_x(0, 0)
            pre_x[(0, 1)] = load_x(0, 1)

            # --- constants ---
            wq_sb = const.tile([128, KT, H_LOCAL * DH], DT)
            nc.sync.dma_start(wq_sb[:], wq_d[:])
            wk_sb = const.tile([128, KT, DH], DT)
            nc.sync.dma_start(wk_sb[:], wk_d[:])
            wv_sb = const.tile([128, KT, DH], DT)
            nc.sync.dma_start(wv_sb[:], wv_d[:])
            cos_sb = const.tile([DH, N], DT)
            nc.sync.dma_start(cos_sb[:], cos_d[:])
            sin_sb = const.tile([DH, N], DT)
            nc.sync.dma_start(sin_sb[:], sin_d[:])
            csq_sb = const.tile([128, H_LOCAL], F32)
            nc.sync.dma_start(csq_sb[:], csq_d[:])
            csk_sb = const.tile([128, 1], F32)
            nc.sync.dma_start(csk_sb[:], csk_d[:])
            csv_sb = const.tile([128, 1], F32)
            nc.sync.dma_start(csv_sb[:], csv_d[:])
            ones_mm = const.tile([128, 1], DT)
            nc.vector.memset(ones_mm, 1.0)
            ident = const.tile([128, 128], DT)
            make_identity(nc, ident)

            # --- PE warmup: wake the HAM clock gate before real work.
            # One accumulation group: back-to-back MMs, no inter-MM sems.
            warm = const.tile([128, 512], DT)
            nc.vector.memset(warm, 0.0)
            wps = ps.tile([128, 512], F32, tag="s", bufs=2)
            for i in range(WARM_MMS):
                nc.tensor.matmul(
                    wps[:], warm[:, 0:128], warm[:],
                    start=(i == 0), stop=(i == WARM_MMS - 1),
                )

            # needed later than the above: out-proj weights + causal mask
            msk_sb = const.tile([128, 128], DT)
            nc.sync.dma_start(msk_sb[:], msk_d[:])
            wo_sb = const.tile([128, H_LOCAL, DIM], DT)
            nc.sync.dma_start(wo_sb[:], wo_d[:])

            # --- persistent activations (per batch for loose cross-phase deps) ---
            qT_b = [
                store.tile([DH, H_LOCAL, N], DT, tag=f"qT{b}", name=f"qT{b}")
                for b in range(B)
            ]
            kT_b = [
                store.tile([DH, N], DT, tag=f"kT{b}", name=f"kT{b}") for b in range(B)
            ]
            v_b = [
                store.tile([128, TT, DH], DT, tag=f"v{b}", name=f"v{b}")
                for b in range(B)
            ]
            aoT_b = [
                store.tile([DH, H_LOCAL, N], DT, tag=f"aoT{b}", name=f"aoT{b}")
                for b in range(B)
            ]

            ln_state = {}

            def ln_stats_a(b, cg):
                """First half of LN stats (tiles 0-1) — split so the DVE queue
                never sees one long stats lump that delays PSUM evictions."""
                xts = load_x(b, cg)
                mr = small.tile([128, 4, 2], F32, tag="mr")
                for t in (0, 1):
                    stats = small.tile([128, 4, 6], F32, tag="stats")
                    for i in range(4):
                        nc.vector.bn_stats(
                            out=stats[:, i, :], in_=xts[t][:, i * 512 : (i + 1) * 512]
                        )
                    nc.vector.bn_aggr(out=mr[:, t, :], in_=stats[:])
                ln_state[(b, cg)] = (xts, mr)

            def ln_stats_b(b, cg):
                """Second half: stats tiles 2-3, rsqrt, row bounce, broadcast.

                Returns (mu_bc, rstd_bc, m1_bc) row-broadcast views [128, CHUNK].
                """
                xts, mr = ln_state.pop((b, cg))
                for t in (2, 3):
                    stats = small.tile([128, 4, 6], F32, tag="stats")
                    for i in range(4):
                        nc.vector.bn_stats(
                            out=stats[:, i, :], in_=xts[t][:, i * 512 : (i + 1) * 512]
                        )
                    nc.vector.bn_aggr(out=mr[:, t, :], in_=stats[:])
                # Newton rsqrt: y = rsqrt(var+eps), y0 = 1.25-0.25v, 4 iters
                mp = small.tile([128, 3, 4], F32, tag="mp")
                ve = small.tile([128, 4], F32, tag="ve")
                nc.vector.tensor_scalar(
                    out=ve[:], in0=mr[:, :, 1], scalar1=EPS, scalar2=None,
                    op0=mybir.AluOpType.add,
                )
                y = small.tile([128, 4], F32, tag="y")
                nc.vector.tensor_scalar(
                    out=y[:], in0=ve[:], scalar1=-0.25, scalar2=1.25,
                    op0=mybir.AluOpType.mult, op1=mybir.AluOpType.add,
                )
                a = small.tile([128, 4], F32, tag="a")
                for it in range(4):
                    nc.vector.tensor_mul(a[:], y[:], y[:])
                    nc.vector.tensor_mul(a[:], a[:], ve[:])
                    nc.vector.tensor_scalar(
                        out=a[:], in0=a[:], scalar1=-0.5, scalar2=1.5,
                        op0=mybir.AluOpType.mult, op1=mybir.AluOpType.add,
                    )
                    dst = mp[:, 1, :] if it == 3 else y[:]
                    nc.vector.tensor_mul(dst, y[:], a[:])
                nc.vector.tensor_copy(mp[:, 0, :], mr[:, :, 0])
                nc.vector.tensor_mul(mp[:, 2, :], mr[:, :, 0], mp[:, 1, :])
                # partition-layout -> row-layout via DRAM bounce (SWDGE),
                # all 3 stats in one row read + one broadcast
                md = drp.tile([128, 3, 4], F32, tag="md")
                nc.gpsimd.dma_start(md[:], mp[:])
                row3 = sm.tile([1, 3 * CHUNK], DT, tag="row3", name=f"row3_{b}_{cg}")
                nc.gpsimd.dma_start(row3[:], md.rearrange("p c t -> c t p"))
                bc3 = bcp.tile(
                    [128, 3, CHUNK], DT, tag="bc3", name=f"bc3_{b}_{cg}"
                )
                nc.gpsimd.partition_broadcast(
                    bc3.rearrange("p c t -> p (c t)"), row3[:]
                )
                return bc3[:, 0, :], bc3[:, 1, :], bc3[:, 2, :]  # mu, rstd, m1

            def rope_evict(dst, src_ps, cs_col, mu_bc, cosp, sinp):
                # dst = ROPE((src - cs*mu) ) * rstd  (rstd folded into cosp/sinp)
                # all on DVE/PE path: ACT stays exp-only so AV never starves
                bt = ropep.tile([DH, CHUNK], DT, tag="bt")
                nc.vector.tensor_scalar(
                    out=bt[:], in0=mu_bc, scalar1=cs_col, scalar2=None,
                    op0=mybir.AluOpType.mult,
                )
                pc = ropep.tile([DH, CHUNK], DT, tag="pc")
                nc.vector.tensor_sub(pc[:], src_ps[:], bt[:])
                rot = ropep.tile([DH, CHUNK], DT, tag="rot")
                nc.vector.tensor_copy(rot[0:64, :], pc[64:128, :])
                nc.vector.tensor_copy(rot[64:128, :], pc[0:64, :])
                tmp = ropep.tile([DH, CHUNK], DT, tag="tmp")
                nc.vector.tensor_mul(tmp[:], pc[:], cosp[:])
                nc.vector.tensor_mul(rot[:], rot[:], sinp[:])
                nc.vector.tensor_add(dst, tmp[:], rot[:])

            def chunk_proj(b, cg, lnr):
                c0 = cg * CHUNK
                mu_bc, rstd_bc, m1_bc = lnr
                xtc = load_xt(b, cg)
                # rstd folded into the rope multipliers (shared by q0,q1,k)
                cosp = ropep.tile([DH, CHUNK], DT, tag="cosp")
                nc.vector.tensor_mul(cosp[:], cos_sb[:, c0 : c0 + CHUNK], rstd_bc)
                sinp = ropep.tile([DH, CHUNK], DT, tag="sinp")
                nc.vector.tensor_mul(sinp[:], sin_sb[:, c0 : c0 + CHUNK], rstd_bc)
                # wave 1: q heads
                qt0 = ps.tile([DH, CHUNK], F32, tag="acc", bufs=3)
                qt1 = ps.tile([DH, CHUNK], F32, tag="acc", bufs=3)
                for kt in range(KT):
                    rhs = xtc[:, kt, :]
                    nc.tensor.matmul(
                        qt0[:], wq_sb[:, kt, 0:128], rhs,
                        start=(kt == 0), stop=(kt == KT - 1),
                    )
                    nc.tensor.matmul(
                        qt1[:], wq_sb[:, kt, 128:256], rhs,
                        start=(kt == 0), stop=(kt == KT - 1),
                    )
                rope_evict(
                    qT_b[b][:, 0, c0 : c0 + CHUNK], qt0, csq_sb[:, 0:1],
                    mu_bc, cosp, sinp,
                )
                rope_evict(
                    qT_b[b][:, 1, c0 : c0 + CHUNK], qt1, csq_sb[:, 1:2],
                    mu_bc, cosp, sinp,
                )
                # wave 2: k, v
                ktp = ps.tile([DH, CHUNK], F32, tag="acc", bufs=3)
                vtp = ps.tile([DH, CHUNK], F32, tag="acc", bufs=3)
                for kt in range(KT):
                    rhs = xtc[:, kt, :]
                    nc.tensor.matmul(
                        ktp[:], wk_sb[:, kt, :], rhs,
                        start=(kt == 0), stop=(kt == KT - 1),
                    )
                    nc.tensor.matmul(
                        vtp[:], wv_sb[:, kt, :], rhs,
                        start=(kt == 0), stop=(kt == KT - 1),
                    )
                rope_evict(
                    kT_b[b][:, c0 : c0 + CHUNK], ktp, csk_sb[:, 0:1],
                    mu_bc, cosp, sinp,
                )
                # v: correct, then PE-transpose to natural [tok, dh] tiles
                vc = vb.tile([DH, CHUNK], DT, tag="vc")
                nc.vector.tensor_mul(vc[:], vtp[:], rstd_bc)
                bv = ropep.tile([DH, CHUNK], DT, tag="bt")
                nc.vector.tensor_scalar(
                    out=bv[:], in0=m1_bc, scalar1=csv_sb[:, 0:1], scalar2=None,
                    op0=mybir.AluOpType.mult,
                )
                vT = vb.tile([DH, CHUNK], DT, tag="vT")
                nc.vector.tensor_sub(vT[:], vc[:], bv[:])
                vn_ps = ps.tile([128, 512], F32, tag="s", bufs=2)
                for tv in range(4):
                    nc.tensor.matmul(
                        vn_ps[:, tv * 128 : (tv + 1) * 128],
                        vT[:, tv * 128 : (tv + 1) * 128],
                        ident[:],
                    )
                nc.scalar.copy(
                    v_b[b][:, cg * 4 : (cg + 1) * 4, :],
                    vn_ps[:].rearrange("p (t d) -> p t d", t=4),
                )

            def attn_group(b, h, qg):
                q0 = qg * QG
                nkt = (qg + 1) * (QG // 128)
                avT = ps.tile([DH, QG], F32, tag="av", bufs=2)
                sums = ps.tile([1, QG], F32, tag="sums", bufs=1)
                for kt in range(nkt):
                    off = max(0, kt * 128 - q0)
                    diag = kt * 128 >= q0
                    st = ps.tile([128, QG], F32, tag="s", bufs=2)
                    nc.tensor.matmul(
                        st[:, off:],
                        kT_b[b][:, kt * 128 : (kt + 1) * 128],
                        qT_b[b][:, h, q0 + off : q0 + QG],
                        start=True,
                        stop=not diag,
                    )
                    if diag:  # causal mask accumulated on the PE itself
                        nc.tensor.matmul(
                            st[:, off : off + 128],
                            ident[:],
                            msk_sb[:],
                            start=False,
                            stop=True,
                        )
                    et = ep.tile([128, QG], DT, tag="et")
                    nc.scalar.activation(
                        out=et[:, off:],
                        in_=st[:, off:],
                        func=mybir.ActivationFunctionType.Exp,
                        scale=SCALE,
                    )
                    nc.tensor.matmul(
                        avT[:, off:],
                        v_b[b][:, kt, :],
                        et[:, off:],
                        start=(kt == 0),
                        stop=(kt == nkt - 1),
                    )
                    nc.tensor.matmul(
                        sums[:, off:],
                        ones_mm[:],
                        et[:, off:],
                        start=(kt == 0),
                        stop=(kt == nkt - 1),
                    )
                # reciprocal on a [128, 4] reshape via DRAM bounce
                # (DVE recip is FD-serial: [1,512] costs ~2.3us, [128,4] ~0.1us)
                ss = sm.tile([1, QG], F32, tag="ss")
                nc.scalar.copy(ss[:], sums[:])
                dr = drp.tile([1, QG], F32, tag="dr")
                nc.gpsimd.dma_start(dr[:], ss[:])
                rt = sm.tile([128, 4], F32, tag="rt")
                nc.gpsimd.dma_start(
                    rt[:], dr.rearrange("o (p f) -> (o p) f", p=128)
                )
                nc.vector.reciprocal(out=rt[:], in_=rt[:])
                dr2 = drp.tile([1, QG], F32, tag="dr2")
                nc.gpsimd.dma_start(
                    dr2.rearrange("o (p f) -> (o p) f", p=128), rt[:]
                )
                rr = sm.tile([1, QG], F32, tag="rr")
                nc.gpsimd.dma_start(rr[:], dr2[:])
                rbc = sm.tile([128, QG], F32, tag="rbc")
                nc.gpsimd.partition_broadcast(rbc[:], rr[:])
                nc.vector.tensor_mul(
                    aoT_b[b][:, h, q0 : q0 + QG], avT[:], rbc[:]
                )

            def outproj_group(b, tt):
                ot = op.tile([128, DIM], DT, tag="ot")
                for dg in range(4):
                    opp = ps.tile([128, 512], F32, tag="acc", bufs=3)
                    for h in range(H_LOCAL):
                        nc.tensor.matmul(
                            opp[:],
                            aoT_b[b][:, h, tt * 128 : (tt + 1) * 128],
                            wo_sb[:, h, dg * 512 : (dg + 1) * 512],
                            start=(h == 0),
                            stop=(h == H_LOCAL - 1),
                        )
                    if dg % 2 == 0:
                        nc.scalar.copy(ot[:, dg * 512 : (dg + 1) * 512], opp[:])
                    else:
                        nc.vector.tensor_copy(ot[:, dg * 512 : (dg + 1) * 512], opp[:])
                nc.sync.dma_start(out_d[b, tt * 128 : (tt + 1) * 128, :], ot[:])

            for _rep in range(repeat):
                # phase 1: b0 projections (stats + xT emitted one chunk ahead)
                lnr = {}
                ln_stats_a(0, 0)
                lnr[(0, 0)] = ln_stats_b(0, 0)
                ln_stats_a(0, 1)
                lnr[(0, 1)] = ln_stats_b(0, 1)
                pre_xt[(0, 1)] = load_xt(0, 1)
                chunk_proj(0, 0, lnr.pop((0, 0)))
                ln_stats_a(0, 2)
                lnr[(0, 2)] = ln_stats_b(0, 2)
                pre_xt[(0, 2)] = load_xt(0, 2)
                chunk_proj(0, 1, lnr.pop((0, 1)))
                ln_stats_a(0, 3)
                lnr[(0, 3)] = ln_stats_b(0, 3)
                pre_xt[(0, 3)] = load_xt(0, 3)
                chunk_proj(0, 2, lnr.pop((0, 2)))
                chunk_proj(0, 3, lnr.pop((0, 3)))
                # phase 2: b0 attention + b0 out-proj + b1 projections
                for qg in range(NQG):
                    pre_xt[(1, qg)] = load_xt(1, qg)
                    ln_stats_a(1, qg)
                    attn_group(0, 0, qg)
                    lnr[(1, qg)] = ln_stats_b(1, qg)
                    attn_group(0, 1, qg)
                    chunk_proj(1, qg, lnr.pop((1, qg)))
                    for tt in range(4 * qg, 4 * qg + 4):
                        outproj_group(0, tt)
                # phase 3: b1 attention + b1 out-proj
                for qg in range(NQG):
                    attn_group(1, 0, qg)
                    attn_group(1, 1, qg)
                    for tt in range(4 * qg, 4 * qg + 4):
                        outproj_group(1, tt)

    nc.compile()
    return nc


def make_in_maps(x, gamma, Wq, Wkv, Wo):
    xbf = np.asarray(x, dtype=np.float32).astype(DT_NP)
    x_nat = np.ascontiguousarray(xbf)
    # xT[p, b, cg, kt, t] = x[b, cg*CHUNK + t, kt*128 + p]
    xT = np.ascontiguousarray(
        xbf.reshape(B, NCH, CHUNK, KT, 128).transpose(4, 0, 1, 3, 2)
    )
    g = np.asarray(gamma, dtype=np.float32)
    Wq = np.asarray(Wq, dtype=np.float32) * g[:, None]
    Wkv = np.asarray(Wkv, dtype=np.float32) * g[:, None]
    Wo = np.asarray(Wo, dtype=np.float32)

    t = np.arange(N, dtype=np.float64)
    inv = 1.0 / (10000.0 ** (np.arange(0, DH, 2, dtype=np.float64) / DH))  # [64]
    fr = np.outer(inv, t)  # [d, t]
    cosT = np.concatenate([np.cos(fr), np.cos(fr)], 0).astype(DT_NP)
    sinT = np.concatenate([-np.sin(fr), np.sin(fr)], 0).astype(DT_NP)
    mask = np.where(
        np.arange(128)[:, None] > np.arange(128)[None, :], NEG, 0.0
    ).astype(DT_NP)

    def pt(w):  # [DIM, M] -> [128, KT, M] partition-major
        return np.ascontiguousarray(
            w.reshape(KT, 128, -1).transpose(1, 0, 2).astype(DT_NP)
        )

    Wk = Wkv[:, :DH]
    Wv = Wkv[:, DH:]
    # column sums for the LN-fold correction (computed in f32 on bf16 weights
    # to match what the device matmul actually contracts)
    csk = np.ascontiguousarray(
        Wk.astype(DT_NP).astype(np.float32).sum(0)[:, None].astype(np.float32)
    )
    csv = np.ascontiguousarray(
        Wv.astype(DT_NP).astype(np.float32).sum(0)[:, None].astype(np.float32)
    )
    maps = []
    for c in range(N_CORES):
        Wq_c = Wq[:, c * H_LOCAL * DH : (c + 1) * H_LOCAL * DH]
        wq_c = pt(Wq_c)
        csq_c = np.ascontiguousarray(
            Wq_c.astype(DT_NP).astype(np.float32).sum(0)
            .reshape(H_LOCAL, DH).T.astype(np.float32)
        )
        wo_c = np.ascontiguousarray(
            Wo[c * H_LOCAL * DH : (c + 1) * H_LOCAL * DH]
            .reshape(H_LOCAL, DH, DIM)
            .transpose(1, 0, 2)
            .astype(DT_NP)
        )
        maps.append(
            {
                "x_in": x_nat,
                "xT": xT,
                "wq": wq_c,
                "wk": pt(Wk),
                "wv": pt(Wv),
                "wo": wo_c,
                "csq": csq_c,
                "csk": csk,
                "csv": csv,
                "cosT": cosT,
                "sinT": sinT,
                "mask": mask,
            }
        )
    return maps


_NC_CACHE = {}


def get_nc(repeat=1, phase=4):
    key = (repeat,)
    if key not in _NC_CACHE:
        _NC_CACHE[key] = build_nc(repeat)
    return _NC_CACHE[key]


def kernel(x, gamma, Wq, Wkv, Wo, _trace=False, _repeat=1):
    from concourse import bass_utils

    nc = get_nc(_repeat)
    in_maps = make_in_maps(x, gamma, Wq, Wkv, Wo)
    res = bass_utils.run_bass_kernel_spmd(
        nc, in_maps, core_ids=list(range(N_CORES)), trace=_trace
    )
    out = np.zeros((B, N, DIM), dtype=np.float32)
    for r in res.results:
        out += np.asarray(r["out_partial"], dtype=np.float32)
    if _trace:
        kernel.last_results = res
    return out


# revision 32
# speedup vs baseline: 1.0624x; 1.0624x over previous
"""MQA attention (LN + QKV proj + RoPE + causal attn + out-proj) on 8 trn2 cores.

Sharding: tensor-parallel over heads (2 heads/core, Wq cols + Wo rows), KV
replicated (single KV head), out-proj produces per-core partial sums that the
host reduces.

v4 per-core dataflow:
  Host ships BOTH x (natural, for LN stats) and xT (transposed, for matmul
  rhs) — no on-device activation transposes (DMA-xbar transposes serialize
  against all other DMA traffic, ~8.5us each; PE transposes burn the
  critical engine).
  LayerNorm is folded into the projections:
      proj(LN(x)) = (W^T x - csum * mu_t) * rstd_t
  with csum = W^T 1 precomputed on host, and mu/rstd per token applied at
  PSUM-evict time (rstd folded into the RoPE cos/sin for q,k).
  rstd comes from a Newton rsqrt on DVE (no ACT sqrt -> single exp table).
  Per-token stats cross from partition-layout to row-layout via a tiny
  DRAM bounce (gpsimd SWDGE), as does the softmax-sum reciprocal.
  Attention: S^T = k @ q^T causal-blocked -> exp on ScalarE -> AV + row
  sums via ones-matmul -> normalize -> out-proj partial -> host sum.
  Emission pipelines b0-attention with b1-projections and out-proj with
  attention to keep the PE dense (HAM stays warm).
"""

import sys

if "/opt/trn_rl_repo" not in sys.path:
    sys.path.insert(0, "/opt/trn_rl_repo")

import ml_dtypes
import numpy as np

import concourse.bass as bass
import concourse.tile as tile
from concourse import bacc, mybir
from concourse.masks import make_identity

F32 = mybir.dt.float32
DT = mybir.dt.bfloat16  # matmul operand storage dtype
DT_NP = ml_dtypes.bfloat16

B, N, DIM, DH, HEADS = 2, 2048, 2048, 128, 16
H_LOCAL = 2  # heads per core
N_CORES = 8
KT = DIM // 128  # k-tiles over the model dim
TT = N // 128  # token tiles per batch
CHUNK = 512  # token chunk for projection phase
NCH = N // CHUNK  # chunks per batch
QG = 512  # q-group width in attention
NQG = N // QG
SCALE = float(DH) ** -0.5
EPS = 1e-5
NEG = -1e30
WARM_MMS = 48


def build_nc(repeat=1):
    nc = bacc.Bacc(None, target_bir_lowering=False, debug=False)

    x_d = nc.dram_tensor("x_in", [B, N, DIM], DT, kind="ExternalInput")
    xt_d = nc.dram_tensor("xT", [128, B, NCH, KT, CHUNK], DT, kind="ExternalInput")
    wq_d = nc.dram_tensor("wq", [128, KT, H_LOCAL * DH], DT, kind="ExternalInput")
    wk_d = nc.dram_tensor("wk", [128, KT, DH], DT, kind="ExternalInput")
    wv_d = nc.dram_tensor("wv", [128, KT, DH], DT, kind="ExternalInput")
    wo_d = nc.dram_tensor("wo", [128, H_LOCAL, DIM], DT, kind="ExternalInput")
    csq_d = nc.dram_tensor("csq", [128, H_LOCAL], F32, kind="ExternalInput")
    csk_d = nc.dram_tensor("csk", [128, 1], F32, kind="ExternalInput")
    csv_d = nc.dram_tensor("csv", [128, 1], F32, kind="ExternalInput")
    cos_d = nc.dram_tensor("cosT", [DH, N], DT, kind="ExternalInput")
    sin_d = nc.dram_tensor("sinT", [DH, N], DT, kind="ExternalInput")
    msk_d = nc.dram_tensor("mask", [128, 128], DT, kind="ExternalInput")
    out_d = nc.dram_tensor("out_partial", [B, N, DIM], DT, kind="ExternalOutput")

    with tile.TileContext(nc) as tc:
        with (
            tc.tile_pool(name="const", bufs=1) as const,
            tc.tile_pool(name="xp", bufs=8) as xp,
            tc.tile_pool(name="xtp", bufs=2) as xtp,
            tc.tile_pool(name="store", bufs=1) as store,
            tc.tile_pool(name="small", bufs=6) as small,
            tc.tile_pool(name="rope", bufs=2) as ropep,
            tc.tile_pool(name="bc", bufs=2) as bcp,
            tc.tile_pool(name="ep", bufs=4) as ep,
            tc.tile_pool(name="sm", bufs=2) as sm,
            tc.tile_pool(name="vb", bufs=2) as vb,
            tc.tile_pool(name="op", bufs=3) as op,
            tc.tile_pool(name="dr", bufs=2, space="DRAM") as drp,
            tc.tile_pool(name="ps", bufs=1, space="PSUM") as ps,
        ):
            # --- first chunk's inputs before the weights so LN starts early ---
            pre_x = {}

            def load_x(b, cg):
                key = (b, cg)
                if key in pre_x:
                    return pre_x.pop(key)
                tiles = []
                c0 = cg * CHUNK
                for t in range(CHUNK // 128):
                    tok0 = c0 + t * 128
                    x_t = xp.tile([128, DIM], DT, tag="x", name=f"x_{b}_{cg}_{t}")
                    nc.gpsimd.dma_start(x_t[:], x_d[b, tok0 : tok0 + 128, :])
                    tiles.append(x_t)
                return tiles

            pre_xt = {}

            def load_xt(b, cg):
                key = (b, cg)
                if key in pre_xt:
                    return pre_xt.pop(key)
                xtc = xtp.tile([128, KT, CHUNK], DT, tag="xtc", name=f"xt_{b}_{cg}")
                nc.gpsimd.dma_start(xtc[:], xt_d[:, b, cg])
                return xtc

            pre_xt[(0, 0)] = load_xt(0, 0)
            pre_x[(0, 0)] = load_x(0, 0)
            pre_x[(0, 1)] = load_x(0, 1)

            # --- constants ---
            wq_sb = const.tile([128, KT, H_LOCAL * DH], DT)
            nc.sync.dma_start(wq_sb[:], wq_d[:])
            wk_sb = const.tile([128, KT, DH], DT)
            nc.sync.dma_start(wk_sb[:], wk_d[:])
            wv_sb = const.tile([128, KT, DH], DT)
            nc.sync.dma_start(wv_sb[:], wv_d[:])
            cos_sb = const.tile([DH, N], DT)
            nc.sync.dma_start(cos_sb[:], cos_d[:])
            sin_sb = const.tile([DH, N], DT)
            nc.sync.dma_start(sin_sb[:], sin_d[:])
            csq_sb = const.tile([128, H_LOCAL], F32)
            nc.sync.dma_start(csq_sb[:], csq_d[:])
            csk_sb = const.tile([128, 1], F32)
            nc.sync.dma_start(csk_sb[:], csk_d[:])
            csv_sb = const.tile([128, 1], F32)
            nc.sync.dma_start(csv_sb[:], csv_d[:])
            ones_mm = const.tile([128, 1], DT)
            nc.vector.memset(ones_mm, 1.0)
            ident = const.tile([128, 128], DT)
            make_identity(nc, ident)

            # --- PE warmup: wake the HAM clock gate before real work.
            # One accumulation group: back-to-back MMs, no inter-MM sems.
            warm = const.tile([128, 512], DT)
            nc.vector.memset(warm, 0.0)
            wps = ps.tile([128, 512], F32, tag="s", bufs=2)
            for i in range(WARM_MMS):
                nc.tensor.matmul(
                    wps[:], warm[:, 0:128], warm[:],
                    start=(i == 0), stop=(i == WARM_MMS - 1),
                )

            # needed later than the above: out-proj weights + causal mask
            msk_sb = const.tile([128, 128], DT)
            nc.sync.dma_start(msk_sb[:], msk_d[:])
            wo_sb = const.tile([128, H_LOCAL, DIM], DT)
            nc.sync.dma_start(wo_sb[:], wo_d[:])

            # --- persistent activations (per batch for loose cross-phase deps) ---
            qT_b = [
                store.tile([DH, H_LOCAL, N], DT, tag=f"qT{b}", name=f"qT{b}")
                for b in range(B)
            ]
            kT_b = [
                store.tile([DH, N], DT, tag=f"kT{b}", name=f"kT{b}") for b in range(B)
            ]
            v_b = [
                store.tile([128, TT, DH], DT, tag=f"v{b}", name=f"v{b}")
                for b in range(B)
            ]
            aoT_b = [
                store.tile([DH, H_LOCAL, N], DT, tag=f"aoT{b}", name=f"aoT{b}")
                for b in range(B)
            ]

            ln_state = {}

            def ln_stats_a(b, cg):
                """First half of LN stats (tiles 0-1) — split so the DVE queue
                never sees one long stats lump that delays PSUM evictions."""
                xts = load_x(b, cg)
                mr = small.tile([128, 4, 2], F32, tag="mr")
                for t in (0, 1):
                    stats = small.tile([128, 4, 6], F32, tag="stats")
                    for i in range(4):
                        nc.vector.bn_stats(
                            out=stats[:, i, :], in_=xts[t][:, i * 512 : (i + 1) * 512]
                        )
                    nc.vector.bn_aggr(out=mr[:, t, :], in_=stats[:])
                ln_state[(b, cg)] = (xts, mr)

            def ln_stats_b(b, cg):
                """Second half: stats tiles 2-3, rsqrt, row bounce, broadcast.

                Returns (mu_bc, rstd_bc, m1_bc) row-broadcast views [128, CHUNK].
                """
                xts, mr = ln_state.pop((b, cg))
                for t in (2, 3):
                    stats = small.tile([128, 4, 6], F32, tag="stats")
                    for i in range(4):
                        nc.vector.bn_stats(
                            out=stats[:, i, :], in_=xts[t][:, i * 512 : (i + 1) * 512]
                        )
                    nc.vector.bn_aggr(out=mr[:, t, :], in_=stats[:])
                # Newton rsqrt: y = rsqrt(var+eps), y0 = 1.25-0.25v, 4 iters
                mp = small.tile([128, 3, 4], F32, tag="mp")
                ve = small.tile([128, 4], F32, tag="ve")
                nc.vector.tensor_scalar(
                    out=ve[:], in0=mr[:, :, 1], scalar1=EPS, scalar2=None,
                    op0=mybir.AluOpType.add,
                )
                y = small.tile([128, 4], F32, tag="y")
                nc.vector.tensor_scalar(
                    out=y[:], in0=ve[:], scalar1=-0.25, scalar2=1.25,
                    op0=mybir.AluOpType.mult, op1=mybir.AluOpType.add,
                )
                a = small.tile([128, 4], F32, tag="a")
                for it in range(4):
                    nc.vector.tensor_mul(a[:], y[:], y[:])
                    nc.vector.tensor_mul(a[:], a[:], ve[:])
                    nc.vector.tensor_scalar(
                        out=a[:], in0=a[:], scalar1=-0.5, scalar2=1.5,
                        op0=mybir.AluOpType.mult, op1=mybir.AluOpType.add,
                    )
                    dst = mp[:, 1, :] if it == 3 else y[:]
                    nc.vector.tensor_mul(dst, y[:], a[:])
                nc.vector.tensor_copy(mp[:, 0, :], mr[:, :, 0])
                nc.vector.tensor_mul(mp[:, 2, :], mr[:, :, 0], mp[:, 1, :])
                # partition-layout -> row-layout via DRAM bounce (SWDGE),
                # all 3 stats in one row read + one broadcast
                md = drp.tile([128, 3, 4], F32, tag="md")
                nc.gpsimd.dma_start(md[:], mp[:])
                row3 = sm.tile([1, 3 * CHUNK], DT, tag="row3", name=f"row3_{b}_{cg}")
                nc.gpsimd.dma_start(row3[:], md.rearrange("p c t -> c t p"))
                bc3 = bcp.tile(
                    [128, 3, CHUNK], DT, tag="bc3", name=f"bc3_{b}_{cg}"
                )
                nc.gpsimd.partition_broadcast(
                    bc3.rearrange("p c t -> p (c t)"), row3[:]
                )
                return bc3[:, 0, :], bc3[:, 1, :], bc3[:, 2, :]  # mu, rstd, m1

            def rope_evict(dst, src_ps, cs_col, mu_bc, cosp, sinp):
                # dst = ROPE((src - cs*mu) ) * rstd  (rstd folded into cosp/sinp)
                bt = ropep.tile([DH, CHUNK], DT, tag="bt")
                nc.scalar.activation(
                    out=bt[:], in_=mu_bc,
                    func=mybir.ActivationFunctionType.Copy, scale=cs_col,
                )
                pc = ropep.tile([DH, CHUNK], DT, tag="pc")
                nc.vector.tensor_sub(pc[:], src_ps[:], bt[:])
                rot = ropep.tile([DH, CHUNK], DT, tag="rot")
                nc.scalar.copy(rot[0:64, :], pc[64:128, :])
                nc.scalar.copy(rot[64:128, :], pc[0:64, :])
                tmp = ropep.tile([DH, CHUNK], DT, tag="tmp")
                nc.vector.tensor_mul(tmp[:], pc[:], cosp[:])
                nc.vector.tensor_mul(rot[:], rot[:], sinp[:])
                nc.vector.tensor_add(dst, tmp[:], rot[:])

            def chunk_proj(b, cg, lnr):
                c0 = cg * CHUNK
                mu_bc, rstd_bc, m1_bc = lnr
                xtc = load_xt(b, cg)
                # rstd folded into the rope multipliers (shared by q0,q1,k)
                cosp = ropep.tile([DH, CHUNK], DT, tag="cosp")
                nc.vector.tensor_mul(cosp[:], cos_sb[:, c0 : c0 + CHUNK], rstd_bc)
                sinp = ropep.tile([DH, CHUNK], DT, tag="sinp")
                nc.vector.tensor_mul(sinp[:], sin_sb[:, c0 : c0 + CHUNK], rstd_bc)
                # wave 1: q heads
                qt0 = ps.tile([DH, CHUNK], F32, tag="acc", bufs=3)
                qt1 = ps.tile([DH, CHUNK], F32, tag="acc", bufs=3)
                for kt in range(KT):
                    rhs = xtc[:, kt, :]
                    nc.tensor.matmul(
                        qt0[:], wq_sb[:, kt, 0:128], rhs,
                        start=(kt == 0), stop=(kt == KT - 1),
                    )
                    nc.tensor.matmul(
                        qt1[:], wq_sb[:, kt, 128:256], rhs,
                        start=(kt == 0), stop=(kt == KT - 1),
                    )
                rope_evict(
                    qT_b[b][:, 0, c0 : c0 + CHUNK], qt0, csq_sb[:, 0:1],
                    mu_bc, cosp, sinp,
                )
                rope_evict(
                    qT_b[b][:, 1, c0 : c0 + CHUNK], qt1, csq_sb[:, 1:2],
                    mu_bc, cosp, sinp,
                )
                # wave 2: k, v
                ktp = ps.tile([DH, CHUNK], F32, tag="acc", bufs=3)
                vtp = ps.tile([DH, CHUNK], F32, tag="acc", bufs=3)
                for kt in range(KT):
                    rhs = xtc[:, kt, :]
                    nc.tensor.matmul(
                        ktp[:], wk_sb[:, kt, :], rhs,
                        start=(kt == 0), stop=(kt == KT - 1),
                    )
                    nc.tensor.matmul(
                        vtp[:], wv_sb[:, kt, :], rhs,
                        start=(kt == 0), stop=(kt == KT - 1),
                    )
                rope_evict(
                    kT_b[b][:, c0 : c0 + CHUNK], ktp, csk_sb[:, 0:1],
                    mu_bc, cosp, sinp,
                )
                # v: correct, then PE-transpose to natural [tok, dh] tiles
                vc = vb.tile([DH, CHUNK], DT, tag="vc")
                nc.vector.tensor_mul(vc[:], vtp[:], rstd_bc)
                bv = ropep.tile([DH, CHUNK], DT, tag="bt")
                nc.scalar.activation(
                    out=bv[:], in_=m1_bc,
                    func=mybir.ActivationFunctionType.Copy, scale=csv_sb[:, 0:1],
                )
                vT = vb.tile([DH, CHUNK], DT, tag="vT")
                nc.vector.tensor_sub(vT[:], vc[:], bv[:])
                vn_ps = ps.tile([128, 512], F32, tag="s", bufs=2)
                for tv in range(4):
                    nc.tensor.matmul(
                        vn_ps[:, tv * 128 : (tv + 1) * 128],
                        vT[:, tv * 128 : (tv + 1) * 128],
                        ident[:],
                    )
                nc.scalar.copy(
                    v_b[b][:, cg * 4 : (cg + 1) * 4, :],
                    vn_ps[:].rearrange("p (t d) -> p t d", t=4),
                )

            def attn_group(b, h, qg):
                q0 = qg * QG
                nkt = (qg + 1) * (QG // 128)
                avT = ps.tile([DH, QG], F32, tag="av", bufs=2)
                sums = ps.tile([1, QG], F32, tag="sums", bufs=1)
                for kt in range(nkt):
                    off = max(0, kt * 128 - q0)
                    diag = kt * 128 >= q0
                    st = ps.tile([128, QG], F32, tag="s", bufs=2)
                    nc.tensor.matmul(
                        st[:, off:],
                        kT_b[b][:, kt * 128 : (kt + 1) * 128],
                        qT_b[b][:, h, q0 + off : q0 + QG],
                        start=True,
                        stop=not diag,
                    )
                    if diag:  # causal mask accumulated on the PE itself
                        nc.tensor.matmul(
                            st[:, off : off + 128],
                            ident[:],
                            msk_sb[:],
                            start=False,
                            stop=True,
                        )
                    et = ep.tile([128, QG], DT, tag="et")
                    nc.scalar.activation(
                        out=et[:, off:],
                        in_=st[:, off:],
                        func=mybir.ActivationFunctionType.Exp,
                        scale=SCALE,
                    )
                    nc.tensor.matmul(
                        avT[:, off:],
                        v_b[b][:, kt, :],
                        et[:, off:],
                        start=(kt == 0),
                        stop=(kt == nkt - 1),
                    )
                    nc.tensor.matmul(
                        sums[:, off:],
                        ones_mm[:],
                        et[:, off:],
                        start=(kt == 0),
                        stop=(kt == nkt - 1),
                    )
                # reciprocal on a [128, 4] reshape via DRAM bounce
                # (DVE recip is FD-serial: [1,512] costs ~2.3us, [128,4] ~0.1us)
                ss = sm.tile([1, QG], F32, tag="ss")
                nc.scalar.copy(ss[:], sums[:])
                dr = drp.tile([1, QG], F32, tag="dr")
                nc.gpsimd.dma_start(dr[:], ss[:])
                rt = sm.tile([128, 4], F32, tag="rt")
                nc.gpsimd.dma_start(
                    rt[:], dr.rearrange("o (p f) -> (o p) f", p=128)
                )
                nc.vector.reciprocal(out=rt[:], in_=rt[:])
                dr2 = drp.tile([1, QG], F32, tag="dr2")
                nc.gpsimd.dma_start(
                    dr2.rearrange("o (p f) -> (o p) f", p=128), rt[:]
                )
                rr = sm.tile([1, QG], F32, tag="rr")
                nc.gpsimd.dma_start(rr[:], dr2[:])
                rbc = sm.tile([128, QG], F32, tag="rbc")
                nc.gpsimd.partition_broadcast(rbc[:], rr[:])
                nc.vector.tensor_mul(
                    aoT_b[b][:, h, q0 : q0 + QG], avT[:], rbc[:]
                )

            def outproj_group(b, tt):
                ot = op.tile([128, DIM], DT, tag="ot")
                for dg in range(4):
                    opp = ps.tile([128, 512], F32, tag="acc", bufs=3)
                    for h in range(H_LOCAL):
                        nc.tensor.matmul(
                            opp[:],
                            aoT_b[b][:, h, tt * 128 : (tt + 1) * 128],
                            wo_sb[:, h, dg * 512 : (dg + 1) * 512],
                            start=(h == 0),
                            stop=(h == H_LOCAL - 1),
                        )
                    if dg % 2 == 0:
                        nc.scalar.copy(ot[:, dg * 512 : (dg + 1) * 512], opp[:])
                    else:
                        nc.vector.tensor_copy(ot[:, dg * 512 : (dg + 1) * 512], opp[:])
                nc.sync.dma_start(out_d[b, tt * 128 : (tt + 1) * 128, :], ot[:])

            for _rep in range(repeat):
                # phase 1: b0 projections (stats + xT emitted one chunk ahead)
                lnr = {}
                ln_stats_a(0, 0)
                lnr[(0, 0)] = ln_stats_b(0, 0)
                ln_stats_a(0, 1)
                lnr[(0, 1)] = ln_stats_b(0, 1)
                pre_xt[(0, 1)] = load_xt(0, 1)
                chunk_proj(0, 0, lnr.pop((0, 0)))
                ln_stats_a(0, 2)
                lnr[(0, 2)] = ln_stats_b(0, 2)
                pre_xt[(0, 2)] = load_xt(0, 2)
                chunk_proj(0, 1, lnr.pop((0, 1)))
                ln_stats_a(0, 3)
                lnr[(0, 3)] = ln_stats_b(0, 3)
                pre_xt[(0, 3)] = load_xt(0, 3)
                chunk_proj(0, 2, lnr.pop((0, 2)))
                chunk_proj(0, 3, lnr.pop((0, 3)))
                # phase 2: b0 attention + b0 out-proj + b1 projections
                for qg in range(NQG):
                    pre_xt[(1, qg)] = load_xt(1, qg)
                    ln_stats_a(1, qg)
                    attn_group(0, 0, qg)
                    lnr[(1, qg)] = ln_stats_b(1, qg)
                    attn_group(0, 1, qg)
                    chunk_proj(1, qg, lnr.pop((1, qg)))
                    for tt in range(4 * qg, 4 * qg + 4):
                        outproj_group(0, tt)
                # phase 3: b1 attention + b1 out-proj
                for qg in range(NQG):
                    attn_group(1, 0, qg)
                    attn_group(1, 1, qg)
                    for tt in range(4 * qg, 4 * qg + 4):
                        outproj_group(1, tt)

    nc.compile()
    return nc


def make_in_maps(x, gamma, Wq, Wkv, Wo):
    xbf = np.asarray(x, dtype=np.float32).astype(DT_NP)
    x_nat = np.ascontiguousarray(xbf)
    # xT[p, b, cg, kt, t] = x[b, cg*CHUNK + t, kt*128 + p]
    xT = np.ascontiguousarray(
        xbf.reshape(B, NCH, CHUNK, KT, 128).transpose(4, 0, 1, 3, 2)
    )
    g = np.asarray(gamma, dtype=np.float32)
    Wq = np.asarray(Wq, dtype=np.float32) * g[:, None]
    Wkv = np.asarray(Wkv, dtype=np.float32) * g[:, None]
    Wo = np.asarray(Wo, dtype=np.float32)

    t = np.arange(N, dtype=np.float64)
    inv = 1.0 / (10000.0 ** (np.arange(0, DH, 2, dtype=np.float64) / DH))  # [64]
    fr = np.outer(inv, t)  # [d, t]
    cosT = np.concatenate([np.cos(fr), np.cos(fr)], 0).astype(DT_NP)
    sinT = np.concatenate([-np.sin(fr), np.sin(fr)], 0).astype(DT_NP)
    mask = np.where(
        np.arange(128)[:, None] > np.arange(128)[None, :], NEG, 0.0
    ).astype(DT_NP)

    def pt(w):  # [DIM, M] -> [128, KT, M] partition-major
        return np.ascontiguousarray(
            w.reshape(KT, 128, -1).transpose(1, 0, 2).astype(DT_NP)
        )

    Wk = Wkv[:, :DH]
    Wv = Wkv[:, DH:]
    # column sums for the LN-fold correction (computed in f32 on bf16 weights
    # to match what the device matmul actually contracts)
    csk = np.ascontiguousarray(
        Wk.astype(DT_NP).astype(np.float32).sum(0)[:, None].astype(np.float32)
    )
    csv = np.ascontiguousarray(
        Wv.astype(DT_NP).astype(np.float32).sum(0)[:, None].astype(np.float32)
    )
    maps = []
    for c in range(N_CORES):
        Wq_c = Wq[:, c * H_LOCAL * DH : (c + 1) * H_LOCAL * DH]
        wq_c = pt(Wq_c)
        csq_c = np.ascontiguousarray(
            Wq_c.astype(DT_NP).astype(np.float32).sum(0)
            .reshape(H_LOCAL, DH).T.astype(np.float32)
        )
        wo_c = np.ascontiguousarray(
            Wo[c * H_LOCAL * DH : (c + 1) * H_LOCAL * DH]
            .reshape(H_LOCAL, DH, DIM)
            .transpose(1, 0, 2)
            .astype(DT_NP)
        )
        maps.append(
            {
                "x_in": x_nat,
                "xT": xT,
                "wq": wq_c,
                "wk": pt(Wk),
                "wv": pt(Wv),
                "wo": wo_c,
                "csq": csq_c,
                "csk": csk,
                "csv": csv,
                "cosT": cosT,
                "sinT": sinT,
                "mask": mask,
            }
        )
    return maps


_NC_CACHE = {}


def get_nc(repeat=1, phase=4):
    key = (repeat,)
    if key not in _NC_CACHE:
        _NC_CACHE[key] = build_nc(repeat)
    return _NC_CACHE[key]


def kernel(x, gamma, Wq, Wkv, Wo, _trace=False, _repeat=1):
    from concourse import bass_utils

    nc = get_nc(_repeat)
    in_maps = make_in_maps(x, gamma, Wq, Wkv, Wo)
    res = bass_utils.run_bass_kernel_spmd(
        nc, in_maps, core_ids=list(range(N_CORES)), trace=_trace
    )
    out = np.zeros((B, N, DIM), dtype=np.float32)
    for r in res.results:
        out += np.asarray(r["out_partial"], dtype=np.float32)
    if _trace:
        kernel.last_results = res
    return out
